# revision 24
# baseline (speedup 1.0000x reference)
"""ExpertGraphConv Trainium2 kernel (bf16 pipeline, host-precomputed params).

Full inputs in, full output out. Shards batch dim (B=8) across 8 NeuronCores;
params replicated. Each core processes 2048 tokens x 8 experts = 16384 rows.

Math per (token t, expert i):
  adj = sigmoid(adjacency_logits); wa, wb = w_msg[:D], w_msg[D:]
  a[t,i] = x[t,i] . wa ; b[t,j] = x[t,j] . wb
  strength[t,i,j] = adj[i,j] * sigmoid(a[t,i]+b[t,j]+b_msg) * (i != j)
  msg[t,i] = sum_j strength[t,i,j] x[t,j]
  out = gelu(msg @ Wn^T + x @ Ws^T + bn + bs)

Host side precomputes everything cheap: x cast to bf16 (the value path was
bf16 anyway -- halves x HBM traffic), Wn^T/Ws^T pre-transposed bf16 packed
as [128, 4*512], wa/wb/bias broadcast to 128 partitions, the block-diagonal
half-adjacency [128,128], and the ab6 constant pattern. Device setup is a
handful of DMAs issued on the scalar engine's DGE queue (parallel with the
sync queue's first x loads) + one identity.

Per 128-row chunk (16 tokens x 8 experts), 9-stage software pipeline; the
score stages work on 2-chunk pairs so every PE operand is produced >= 1
full iteration before the PE instruction that consumes it:
  A(c):   DMA xbf (bf16 natural layout)
  C(c):   DVE a/b row-sums (fused mult + row-sum accum) -> ab6 pair tile
          (chunk j of the pair at cols 64j+0 (b) / 64j+33 (a); 64-col
          stride because engine PSUM/SBUF partition accesses must start
          32-aligned)
  D(g):   per pair: PE transpose of the [128,128] ab6 pair tile
  CP(g):  per pair: assemble the K=35 fused-outer operands from abT PSUM
          (DVE: lhs35 rows 0:3/32:35 <- b-groups; ACT: rhs35 <- a-groups;
          rows 3..31 stay zero) -- walrus requires fmap/weight at the
          same SB base partition, hence the copies
  E(g):   per pair: ONE K=35 PE matmul -> scores for both chunks
          [128,256], ACT tanh (sigmoid identity)
  E2(c):  DVE strength: sb = (th+1)*hadj (= sigmoid * adj^T, masked)
  F1(c):  PE 4 bf16 matmuls (lhsT=xbf d-slices, rhs=[Sblk|I128b]) yields
          msg^T and x^T together in PSUM; ACT copy -> bf16 zt
  F2(c):  8 accumulating bf16 matmuls (msg^T vs wnT, x^T vs wsT;
          start=True on the first -- no bias matmul), DVE adds bn+bs into
          PSUM in-place, native Gelu from PSUM, DMA out.
Steady-state engine busy per chunk (measured): PE 2.37us, DVE 2.31us,
ACT 2.26us -- near-balanced saturation; PE floor is the bf16 GEMM work
(F2 4096c + F1 1024c at 2.4GHz).
Dead ends (measured/verified, do not revisit):
 - DMA-XBAR transposes for x^T: ~145ns/16x128-tile descriptor generation
   on the issuing sequencer (~4.6us/chunk), regardless of batching.
 - fp8 e4m3 DoubleRow for the GEMMs: rel err 3.5-5e-2 > 2e-2 tolerance
   (numpy-validated), even with hi/lo residual splits.
 - gpsimd scalar_tensor_tensor has no Pool lowering in walrus
   (tensor_tensor does, but gpsimd shares its SBUF port with DVE and
   ~1.3us software latency stalls the chain -- measured slower).
 - Matmul operands at different SB base partitions are rejected by
   walrus even with tile_position.
 - Engine (DVE/ACT) partition accesses must start at 0/32/64/96.
"""

import math
from contextlib import ExitStack

import ml_dtypes
import numpy as np

import concourse.bacc as bacc
import concourse.mybir as mybir
import concourse.tile as tile
from concourse import bass_utils
from concourse.masks import make_identity

F32 = mybir.dt.float32
BF16 = mybir.dt.bfloat16
AF = mybir.ActivationFunctionType
OP = mybir.AluOpType

B, L, E, D = 8, 2048, 8, 512
N_CORES = 8
P = 128
ROWS_PER_CORE = (B // N_CORES) * L * E  # 16384
NSC = 12  # rotation depth of the scat rotating buffers
NG = 2    # chunks per ab6 transpose group (64-col stride: PSUM reads must
          # start at 32-aligned partitions, so per-chunk groups sit at
          # 64j and 64j+32)
NSG = 6   # rotation depth of ab6 group buffers


def build_nc(n_rows=ROWS_PER_CORE, gelu_tanh_standin=False):
    assert n_rows % P == 0
    n_chunks = n_rows // P
    nd = D // P  # 4 d-chunks

    nc = bacc.Bacc(
        "TRN2", target_bir_lowering=False, debug=False, num_devices=N_CORES
    )

    x_dram = nc.dram_tensor("expert_features", [n_rows, D], BF16, kind="ExternalInput").ap()
    wnt_dram = nc.dram_tensor("wnt", [P, nd * D], BF16, kind="ExternalInput").ap()
    wst_dram = nc.dram_tensor("wst", [P, nd * D], BF16, kind="ExternalInput").ap()
    wab_dram = nc.dram_tensor("wab", [P, 2 * D], BF16, kind="ExternalInput").ap()
    bias_dram = nc.dram_tensor("bias_rep", [P, D], F32, kind="ExternalInput").ap()
    hadj_dram = nc.dram_tensor("hadj", [P, P], BF16, kind="ExternalInput").ap()
    ab6i_dram = nc.dram_tensor("ab6i", [P, NSG * P], BF16, kind="ExternalInput").ap()
    scati_dram = nc.dram_tensor("scati", [P, NSC * 2 * P], BF16, kind="ExternalInput").ap()
    out_dram = nc.dram_tensor("out", [n_rows, D], F32, kind="ExternalOutput").ap()

    with tile.TileContext(nc) as tc, ExitStack() as ctx:
        # ---- static SBUF tensors ----
        I128b = nc.alloc_sbuf_tensor("c_I128b", [P, P], BF16).ap()
        wab_all = nc.alloc_sbuf_tensor("c_wab", [P, 2 * D], BF16).ap()
        wa_rep = wab_all[:, 0:D]
        wb_rep = wab_all[:, D : 2 * D]
        bias_rep = nc.alloc_sbuf_tensor("c_bias_rep", [P, D], F32).ap()
        hadj_blk = nc.alloc_sbuf_tensor("c_hadj_blk", [P, P], BF16).ap()
        wnT_all = nc.alloc_sbuf_tensor("c_wnT", [P, nd * D], BF16).ap()
        wsT_all = nc.alloc_sbuf_tensor("c_wsT", [P, nd * D], BF16).ap()
        wnT = [wnT_all[:, dc * D : (dc + 1) * D] for dc in range(nd)]
        wsT = [wsT_all[:, dc * D : (dc + 1) * D] for dc in range(nd)]
        scat_all = nc.alloc_sbuf_tensor("c_scat", [P, NSC * 2 * P], BF16).ap()
        scat = [scat_all[:, i * 2 * P : (i + 1) * 2 * P] for i in range(NSC)]
        # 2-chunk groups; per chunk j: col 64j+0=b, +1=1, +2=bmsg, +32=1,
        # +33=a, +34=1
        ab6_all = nc.alloc_sbuf_tensor("c_ab6", [P, NSG * P], BF16).ap()
        ab6_g = [ab6_all[:, i * P : (i + 1) * P] for i in range(NSG)]
        # K=35 fused outer-pair operands (rows 3..31 stay zero; per-pair
        # copies rewrite rows 0:3 and 32:35 only)
        lhs35_s = [
            nc.alloc_sbuf_tensor(f"c_lhs35_{i}", [35, P], BF16).ap()
            for i in range(NSG)
        ]
        rhs35_s = [
            nc.alloc_sbuf_tensor(f"c_rhs35_{i}", [35, 2 * P], BF16).ap()
            for i in range(NSG)
        ]

        make_identity(nc, I128b)
        for t_ in lhs35_s:
            nc.gpsimd.memset(t_, 0.0)
        for t_ in rhs35_s:
            nc.gpsimd.memset(t_, 0.0)

        # ---- setup: DMA loads of host-precomputed params (scalar queue),
        # ordered by when the pipeline first needs them ----
        nc.scalar.dma_start(wab_all, wab_dram[:])
        nc.scalar.dma_start(ab6_all, ab6i_dram[:])
        nc.scalar.dma_start(hadj_blk, hadj_dram[:])
        nc.scalar.dma_start(scat_all, scati_dram[:])
        nc.scalar.dma_start(wnT_all, wnt_dram[:])
        nc.scalar.dma_start(wsT_all, wst_dram[:])
        nc.scalar.dma_start(bias_rep, bias_dram[:])

        # ---- main loop (software-pipelined, 7-stage skew) ----
        with (
            tc.tile_pool(name="xp", bufs=10) as xp,
            tc.tile_pool(name="sc", bufs=4) as scp,
            tc.tile_pool(name="small", bufs=3) as smp,
            tc.tile_pool(name="mid", bufs=4) as midp,
            tc.tile_pool(name="ztp", bufs=4) as ztp,
            tc.tile_pool(name="op", bufs=5) as op_,
            tc.tile_pool(name="ps_t", bufs=2, space="PSUM") as ps_t,
            tc.tile_pool(name="ps_o", bufs=1, space="PSUM") as ps_o,
            tc.tile_pool(name="ps_c", bufs=1, space="PSUM") as ps_c,
            tc.tile_pool(name="ps_b", bufs=3, space="PSUM") as ps_b,
        ):
            xins = {}
            abTs = {}
            cps = {}
            zts = {}
            ths = {}

            def stage_a(c):
                rows = slice(c * P, (c + 1) * P)
                xin = xp.tile([P, D], BF16, tag="xin")
                nc.sync.dma_start(xin[:], x_dram[rows, :])
                xins[c] = xin

            def stage_c(c):
                ab6 = ab6_g[(c // NG) % NSG]
                j = c % NG
                xbf = xins[c]
                scr = scp.tile([P, D], BF16, tag="scr")
                nc.vector.scalar_tensor_tensor(
                    out=scr[:], in0=xbf[:], scalar=0.0, in1=wb_rep,
                    op0=OP.bypass, op1=OP.mult, accum_out=ab6[:, 64 * j : 64 * j + 1],
                )
                scr2 = scp.tile([P, D], BF16, tag="scr")
                nc.vector.scalar_tensor_tensor(
                    out=scr2[:], in0=xbf[:], scalar=0.0, in1=wa_rep,
                    op0=OP.bypass, op1=OP.mult,
                    accum_out=ab6[:, 64 * j + 33 : 64 * j + 34],
                )

            def stage_d(c):
                # fires on the last chunk of each group (or the final chunk)
                if not (c % NG == NG - 1 or c == n_chunks - 1):
                    return
                g = c // NG
                abT = ps_t.tile([P, P], BF16, tag="abT")
                nc.tensor.transpose(abT[:], ab6_g[g % NSG][:], I128b)
                abTs[g] = abT

            def stage_cp(c):
                # operand copies one iteration ahead of the fused outer
                # matmul, so the PE never waits on same-iteration
                # cross-engine operands. Split DVE/ACT to balance load.
                if c % NG != NG - 1:
                    return
                g = c // NG
                abT = abTs.pop(g)
                l35 = lhs35_s[g % NSG]
                r35 = rhs35_s[g % NSG]
                nc.vector.tensor_copy(l35[0:3, :], abT[0:3, :])
                nc.vector.tensor_copy(l35[32:35, :], abT[64:67, :])
                nc.scalar.copy(r35[0:3, 0:P], abT[32:35, :])
                nc.scalar.copy(r35[32:35, P : 2 * P], abT[96:99, :])

            def stage_e(c):
                # K=35 fused outer for the chunk pair: scores for chunk 2g
                # in cols 0:128, chunk 2g+1 in cols 128:256
                if c % NG != NG - 1:
                    return
                g = c // NG
                outer = ps_o.tile([P, 2 * P], F32, tag="outer")
                nc.tensor.matmul(
                    outer[:], lhsT=lhs35_s[g % NSG][:], rhs=rhs35_s[g % NSG][:]
                )
                th = midp.tile([P, 2 * P], BF16, tag="th")
                nc.scalar.activation(th[:], outer[:], AF.Tanh, scale=0.5)
                ths[g] = th

            def stage_e_dve(c):
                # sb = (th+1)*hadj = sigmoid*adjT masked
                g, j = c // NG, c % NG
                th = ths[g]
                if j == NG - 1:
                    ths.pop(g)
                sb = scat[c % NSC]
                nc.vector.scalar_tensor_tensor(
                    out=sb[:, 0:P], in0=th[:, j * P : (j + 1) * P], scalar=1.0,
                    in1=hadj_blk, op0=OP.add, op1=OP.mult,
                )

            def stage_f1(c):
                xin = xins.pop(c)
                sb = scat[c % NSC]
                cmb = ps_c.tile([P, 2 * P * nd], F32, tag="cmb")
                for dc in range(nd):
                    nc.tensor.matmul(
                        cmb[:, 2 * P * dc : 2 * P * (dc + 1)],
                        lhsT=xin[:, dc * P : (dc + 1) * P],
                        rhs=sb[:],
                    )
                zt = ztp.tile([P, 2 * P * nd], BF16, tag="zt")
                nc.scalar.copy(zt[:], cmb[:])
                zts[c] = zt

            def stage_f2(c):
                rows = slice(c * P, (c + 1) * P)
                zt = zts.pop(c)
                big = ps_b.tile([P, D], F32, tag="big")
                for dc in range(nd):
                    nc.tensor.matmul(
                        big[:],
                        lhsT=zt[:, 2 * P * dc : 2 * P * dc + P],
                        rhs=wnT[dc],
                        start=(dc == 0),
                        stop=False,
                    )
                for dc in range(nd):
                    nc.tensor.matmul(
                        big[:],
                        lhsT=zt[:, 2 * P * dc + P : 2 * P * (dc + 1)],
                        rhs=wsT[dc],
                        start=False,
                        stop=(dc == nd - 1),
                    )
                # bias into PSUM in-place, then gelu reads the biased PSUM
                nc.vector.scalar_tensor_tensor(
                    out=big[:], in0=big[:], scalar=0.0, in1=bias_rep,
                    op0=OP.bypass, op1=OP.add,
                )
                osb = op_.tile([P, D], F32, tag="osb")
                nc.scalar.activation(
                    osb[:], big[:],
                    AF.Tanh if gelu_tanh_standin else AF.Gelu, scale=1.0,
                )
                nc.sync.dma_start(out_dram[rows, :], osb[:])

            stage_offsets = [
                (stage_a, 0), (stage_c, 1), (stage_d, 2), (stage_cp, 3),
                (stage_e, 4), (stage_e_dve, 6), (stage_f1, 7), (stage_f2, 8),
            ]
            n_stages = 9
            for i in range(n_chunks + n_stages - 1):
                for fn, off in stage_offsets:
                    c = i - off
                    if 0 <= c < n_chunks:
                        fn(c)

    nc.compile()
    return nc


_CACHE = {}


def _get_nc():
    if "nc" not in _CACHE:
        _CACHE["nc"] = build_nc()
    return _CACHE["nc"]


def _make_in_maps(inputs):
    x = np.ascontiguousarray(
        np.asarray(inputs["expert_features"], np.float32)
    ).astype(ml_dtypes.bfloat16)
    assert x.shape == (B, L, E, D)
    shards = x.reshape(N_CORES, ROWS_PER_CORE, D)

    wn = np.asarray(inputs["W_neighbor"], np.float64)
    ws = np.asarray(inputs["W_self"], np.float64)
    bn = np.asarray(inputs["b_neighbor"], np.float64)
    bs = np.asarray(inputs["b_self"], np.float64)
    wmsg = np.asarray(inputs["w_msg"], np.float32)
    bmsg = float(np.asarray(inputs["b_msg"], np.float32).reshape(-1)[0])
    adjL = np.asarray(inputs["adjacency_logits"], np.float64)

    nd = D // P

    def pack_t(w):
        # [128, 4*512]: block dc holds W^T[128dc:128(dc+1), :]
        wt = np.ascontiguousarray(w.T)  # [d, o]
        return (
            wt.reshape(nd, P, D).transpose(1, 0, 2).reshape(P, nd * D)
        ).astype(ml_dtypes.bfloat16)

    wab = np.empty((P, 2 * D), np.float32)
    wab[:, 0:D] = wmsg[:D]
    wab[:, D:] = wmsg[D:]
    bias_rep = np.ascontiguousarray(
        np.broadcast_to((bn + bs).astype(np.float32), (P, D))
    ).astype(np.float32)

    # block-diag half adjacency: h8[p,q] = 0.5*sigmoid(adjL[q,p])*(p!=q)
    adjs = 1.0 / (1.0 + np.exp(-adjL))
    h8 = 0.5 * adjs.T * (1.0 - np.eye(E))
    hadj = np.kron(np.eye(P // E), h8).astype(ml_dtypes.bfloat16)

    ab6i = np.zeros((P, P), np.float32)
    for j in range(NG):
        ab6i[:, 64 * j + 1] = 1.0
        ab6i[:, 64 * j + 2] = bmsg
        ab6i[:, 64 * j + 32] = 1.0
        ab6i[:, 64 * j + 34] = 1.0
    ab6i = np.tile(ab6i, (1, NSG)).astype(ml_dtypes.bfloat16)

    scati = np.zeros((P, NSC * 2 * P), np.float32)
    eye = np.eye(P, dtype=np.float32)
    for i in range(NSC):
        scati[:, i * 2 * P + P : (i + 1) * 2 * P] = eye
    scati = scati.astype(ml_dtypes.bfloat16)

    params = {
        "wnt": pack_t(wn),
        "wst": pack_t(ws),
        "wab": wab.astype(ml_dtypes.bfloat16),
        "bias_rep": bias_rep,
        "hadj": np.ascontiguousarray(hadj),
        "ab6i": np.ascontiguousarray(ab6i),
        "scati": np.ascontiguousarray(scati),
    }
    return [dict(expert_features=shards[c], **params) for c in range(N_CORES)]


def _run(inputs, trace=False):
    nc = _get_nc()
    in_maps = _make_in_maps(inputs)
    res = bass_utils.run_bass_kernel_spmd(
        nc, in_maps, core_ids=list(range(N_CORES)), trace=trace
    )
    out = np.stack([res.results[c]["out"] for c in range(N_CORES)], axis=0)
    return out.reshape(B, L, E, D), res


def kernel(**inputs):
    out, _ = _run(inputs, trace=False)
    return out


# revision 25
# speedup vs baseline: 1.0001x; 1.0001x over previous
"""ExpertGraphConv Trainium2 kernel (bf16 pipeline, host-precomputed params).

Full inputs in, full output out. Shards batch dim (B=8) across 8 NeuronCores;
params replicated. Each core processes 2048 tokens x 8 experts = 16384 rows.

Math per (token t, expert i):
  adj = sigmoid(adjacency_logits); wa, wb = w_msg[:D], w_msg[D:]
  a[t,i] = x[t,i] . wa ; b[t,j] = x[t,j] . wb
  strength[t,i,j] = adj[i,j] * sigmoid(a[t,i]+b[t,j]+b_msg) * (i != j)
  msg[t,i] = sum_j strength[t,i,j] x[t,j]
  out = gelu(msg @ Wn^T + x @ Ws^T + bn + bs)

Host side precomputes everything cheap: x cast to bf16 (the value path was
bf16 anyway -- halves x HBM traffic), Wn^T/Ws^T pre-transposed bf16 packed
as [128, 4*512], wa/wb/bias broadcast to 128 partitions, the block-diagonal
half-adjacency [128,128], and the ab6 constant pattern. Device setup is a
handful of DMAs issued on the scalar engine's DGE queue (parallel with the
sync queue's first x loads) + one identity.

Per 128-row chunk (16 tokens x 8 experts), 9-stage software pipeline; the
score stages work on 2-chunk pairs so every PE operand is produced >= 1
full iteration before the PE instruction that consumes it:
  A(c):   DMA xbf (bf16 natural layout)
  C(c):   DVE a/b row-sums (fused mult + row-sum accum) -> ab6 pair tile
          (chunk j of the pair at cols 64j+0 (b) / 64j+33 (a); 64-col
          stride because engine PSUM/SBUF partition accesses must start
          32-aligned)
  D(g):   per pair: PE transpose of the [128,128] ab6 pair tile
  CP(g):  per pair: assemble the K=35 fused-outer operands from abT PSUM
          (DVE: lhs35 rows 0:3/32:35 <- b-groups; ACT: rhs35 <- a-groups;
          rows 3..31 stay zero) -- walrus requires fmap/weight at the
          same SB base partition, hence the copies
  E(g):   per pair: ONE K=35 PE matmul -> scores for both chunks
          [128,256], ACT tanh (sigmoid identity)
  E2(c):  DVE strength: sb = (th+1)*hadj (= sigmoid * adj^T, masked)
  F1(c):  PE 4 bf16 matmuls (lhsT=xbf d-slices, rhs=[Sblk|I128b]) yields
          msg^T and x^T together in PSUM; ACT copy -> bf16 zt
  F2(c):  8 accumulating bf16 matmuls (msg^T vs wnT, x^T vs wsT;
          start=True on the first -- no bias matmul), DVE adds bn+bs into
          PSUM in-place, native Gelu from PSUM, DMA out.
Steady-state engine busy per chunk (measured): PE 2.37us, DVE 2.31us,
ACT 2.26us -- near-balanced saturation; PE floor is the bf16 GEMM work
(F2 4096c + F1 1024c at 2.4GHz).
Dead ends (measured/verified, do not revisit):
 - DMA-XBAR transposes for x^T: ~145ns/16x128-tile descriptor generation
   on the issuing sequencer (~4.6us/chunk), regardless of batching.
 - fp8 e4m3 DoubleRow for the GEMMs: rel err 3.5-5e-2 > 2e-2 tolerance
   (numpy-validated), even with hi/lo residual splits.
 - gpsimd scalar_tensor_tensor has no Pool lowering in walrus
   (tensor_tensor does, but gpsimd shares its SBUF port with DVE and
   ~1.3us software latency stalls the chain -- measured slower).
 - Matmul operands at different SB base partitions are rejected by
   walrus even with tile_position.
 - Engine (DVE/ACT) partition accesses must start at 0/32/64/96.
"""

import math
from contextlib import ExitStack

import ml_dtypes
import numpy as np

import concourse.bacc as bacc
import concourse.mybir as mybir
import concourse.tile as tile
from concourse import bass_utils
from concourse.masks import make_identity

F32 = mybir.dt.float32
BF16 = mybir.dt.bfloat16
AF = mybir.ActivationFunctionType
OP = mybir.AluOpType

B, L, E, D = 8, 2048, 8, 512
N_CORES = 8
P = 128
ROWS_PER_CORE = (B // N_CORES) * L * E  # 16384
NSC = 8   # rotation depth of the scat rotating buffers
NG = 2    # chunks per ab6 transpose group (64-col stride: PSUM reads must
          # start at 32-aligned partitions, so per-chunk groups sit at
          # 64j and 64j+32)
NSG = 4   # rotation depth of ab6 group buffers


def build_nc(n_rows=ROWS_PER_CORE, gelu_tanh_standin=False):
    assert n_rows % P == 0
    n_chunks = n_rows // P
    nd = D // P  # 4 d-chunks

    nc = bacc.Bacc(
        "TRN2", target_bir_lowering=False, debug=False, num_devices=N_CORES
    )

    x_dram = nc.dram_tensor("expert_features", [n_rows, D], BF16, kind="ExternalInput").ap()
    wnt_dram = nc.dram_tensor("wnt", [P, nd * D], BF16, kind="ExternalInput").ap()
    wst_dram = nc.dram_tensor("wst", [P, nd * D], BF16, kind="ExternalInput").ap()
    wab_dram = nc.dram_tensor("wab", [P, 2 * D], BF16, kind="ExternalInput").ap()
    bias_dram = nc.dram_tensor("bias_rep", [P, D], F32, kind="ExternalInput").ap()
    hadj_dram = nc.dram_tensor("hadj", [P, P], BF16, kind="ExternalInput").ap()
    ab6i_dram = nc.dram_tensor("ab6i", [P, NSG * P], BF16, kind="ExternalInput").ap()
    scati_dram = nc.dram_tensor("scati", [P, NSC * 2 * P], BF16, kind="ExternalInput").ap()
    out_dram = nc.dram_tensor("out", [n_rows, D], F32, kind="ExternalOutput").ap()

    with tile.TileContext(nc) as tc, ExitStack() as ctx:
        # ---- static SBUF tensors ----
        I128b = nc.alloc_sbuf_tensor("c_I128b", [P, P], BF16).ap()
        wab_all = nc.alloc_sbuf_tensor("c_wab", [P, 2 * D], BF16).ap()
        wa_rep = wab_all[:, 0:D]
        wb_rep = wab_all[:, D : 2 * D]
        bias_rep = nc.alloc_sbuf_tensor("c_bias_rep", [P, D], F32).ap()
        hadj_blk = nc.alloc_sbuf_tensor("c_hadj_blk", [P, P], BF16).ap()
        wnT_all = nc.alloc_sbuf_tensor("c_wnT", [P, nd * D], BF16).ap()
        wsT_all = nc.alloc_sbuf_tensor("c_wsT", [P, nd * D], BF16).ap()
        wnT = [wnT_all[:, dc * D : (dc + 1) * D] for dc in range(nd)]
        wsT = [wsT_all[:, dc * D : (dc + 1) * D] for dc in range(nd)]
        scat_all = nc.alloc_sbuf_tensor("c_scat", [P, NSC * 2 * P], BF16).ap()
        scat = [scat_all[:, i * 2 * P : (i + 1) * 2 * P] for i in range(NSC)]
        # 2-chunk groups; per chunk j: col 64j+0=b, +1=1, +2=bmsg, +32=1,
        # +33=a, +34=1
        ab6_all = nc.alloc_sbuf_tensor("c_ab6", [P, NSG * P], BF16).ap()
        ab6_g = [ab6_all[:, i * P : (i + 1) * P] for i in range(NSG)]
        # K=35 fused outer-pair operands (rows 3..31 stay zero; per-pair
        # copies rewrite rows 0:3 and 32:35 only)
        lhs35_s = [
            nc.alloc_sbuf_tensor(f"c_lhs35_{i}", [35, P], BF16).ap()
            for i in range(NSG)
        ]
        rhs35_s = [
            nc.alloc_sbuf_tensor(f"c_rhs35_{i}", [35, 2 * P], BF16).ap()
            for i in range(NSG)
        ]

        make_identity(nc, I128b)
        for t_ in lhs35_s:
            nc.gpsimd.memset(t_, 0.0)
        for t_ in rhs35_s:
            nc.gpsimd.memset(t_, 0.0)

        # ---- setup: DMA loads of host-precomputed params (scalar queue),
        # ordered by when the pipeline first needs them ----
        nc.scalar.dma_start(wab_all, wab_dram[:])
        nc.scalar.dma_start(ab6_all, ab6i_dram[:])
        nc.scalar.dma_start(hadj_blk, hadj_dram[:])
        nc.scalar.dma_start(scat_all, scati_dram[:])
        nc.scalar.dma_start(wnT_all, wnt_dram[:])
        nc.scalar.dma_start(wsT_all, wst_dram[:])
        nc.scalar.dma_start(bias_rep, bias_dram[:])

        # ---- main loop (software-pipelined, 7-stage skew) ----
        with (
            tc.tile_pool(name="xp", bufs=10) as xp,
            tc.tile_pool(name="sc", bufs=4) as scp,
            tc.tile_pool(name="small", bufs=3) as smp,
            tc.tile_pool(name="mid", bufs=3) as midp,
            tc.tile_pool(name="ztp", bufs=4) as ztp,
            tc.tile_pool(name="op", bufs=5) as op_,
            tc.tile_pool(name="ps_t", bufs=2, space="PSUM") as ps_t,
            tc.tile_pool(name="ps_o", bufs=1, space="PSUM") as ps_o,
            tc.tile_pool(name="ps_c", bufs=1, space="PSUM") as ps_c,
            tc.tile_pool(name="ps_b", bufs=3, space="PSUM") as ps_b,
        ):
            xins = {}
            abTs = {}
            cps = {}
            zts = {}
            ths = {}

            def stage_a(c):
                rows = slice(c * P, (c + 1) * P)
                xin = xp.tile([P, D], BF16, tag="xin")
                nc.sync.dma_start(xin[:], x_dram[rows, :])
                xins[c] = xin

            def stage_c(c):
                ab6 = ab6_g[(c // NG) % NSG]
                j = c % NG
                xbf = xins[c]
                scr = scp.tile([P, D], BF16, tag="scr")
                nc.vector.scalar_tensor_tensor(
                    out=scr[:], in0=xbf[:], scalar=0.0, in1=wb_rep,
                    op0=OP.bypass, op1=OP.mult, accum_out=ab6[:, 64 * j : 64 * j + 1],
                )
                scr2 = scp.tile([P, D], BF16, tag="scr")
                nc.vector.scalar_tensor_tensor(
                    out=scr2[:], in0=xbf[:], scalar=0.0, in1=wa_rep,
                    op0=OP.bypass, op1=OP.mult,
                    accum_out=ab6[:, 64 * j + 33 : 64 * j + 34],
                )

            def stage_d(c):
                # fires on the last chunk of each group (or the final chunk)
                if not (c % NG == NG - 1 or c == n_chunks - 1):
                    return
                g = c // NG
                abT = ps_t.tile([P, P], BF16, tag="abT")
                nc.tensor.transpose(abT[:], ab6_g[g % NSG][:], I128b)
                abTs[g] = abT

            def stage_cp(c):
                # operand copies one iteration ahead of the fused outer
                # matmul, so the PE never waits on same-iteration
                # cross-engine operands. Split DVE/ACT to balance load.
                if c % NG != NG - 1:
                    return
                g = c // NG
                abT = abTs.pop(g)
                l35 = lhs35_s[g % NSG]
                r35 = rhs35_s[g % NSG]
                nc.vector.tensor_copy(l35[0:3, :], abT[0:3, :])
                nc.vector.tensor_copy(l35[32:35, :], abT[64:67, :])
                nc.scalar.copy(r35[0:3, 0:P], abT[32:35, :])
                nc.scalar.copy(r35[32:35, P : 2 * P], abT[96:99, :])

            def stage_e(c):
                # K=35 fused outer for the chunk pair: scores for chunk 2g
                # in cols 0:128, chunk 2g+1 in cols 128:256
                if c % NG != NG - 1:
                    return
                g = c // NG
                outer = ps_o.tile([P, 2 * P], F32, tag="outer")
                nc.tensor.matmul(
                    outer[:], lhsT=lhs35_s[g % NSG][:], rhs=rhs35_s[g % NSG][:]
                )
                th = midp.tile([P, 2 * P], BF16, tag="th")
                nc.scalar.activation(th[:], outer[:], AF.Tanh, scale=0.5)
                ths[g] = th

            def stage_e_dve(c):
                # sb = (th+1)*hadj = sigmoid*adjT masked
                g, j = c // NG, c % NG
                th = ths[g]
                if j == NG - 1:
                    ths.pop(g)
                sb = scat[c % NSC]
                nc.vector.scalar_tensor_tensor(
                    out=sb[:, 0:P], in0=th[:, j * P : (j + 1) * P], scalar=1.0,
                    in1=hadj_blk, op0=OP.add, op1=OP.mult,
                )

            def stage_f1(c):
                xin = xins.pop(c)
                sb = scat[c % NSC]
                cmb = ps_c.tile([P, 2 * P * nd], F32, tag="cmb")
                for dc in range(nd):
                    nc.tensor.matmul(
                        cmb[:, 2 * P * dc : 2 * P * (dc + 1)],
                        lhsT=xin[:, dc * P : (dc + 1) * P],
                        rhs=sb[:],
                    )
                zt = ztp.tile([P, 2 * P * nd], BF16, tag="zt")
                nc.scalar.copy(zt[:], cmb[:])
                zts[c] = zt

            def stage_f2(c):
                rows = slice(c * P, (c + 1) * P)
                zt = zts.pop(c)
                big = ps_b.tile([P, D], F32, tag="big")
                for dc in range(nd):
                    nc.tensor.matmul(
                        big[:],
                        lhsT=zt[:, 2 * P * dc : 2 * P * dc + P],
                        rhs=wnT[dc],
                        start=(dc == 0),
                        stop=False,
                    )
                for dc in range(nd):
                    nc.tensor.matmul(
                        big[:],
                        lhsT=zt[:, 2 * P * dc + P : 2 * P * (dc + 1)],
                        rhs=wsT[dc],
                        start=False,
                        stop=(dc == nd - 1),
                    )
                # bias into PSUM in-place, then gelu reads the biased PSUM
                nc.vector.scalar_tensor_tensor(
                    out=big[:], in0=big[:], scalar=0.0, in1=bias_rep,
                    op0=OP.bypass, op1=OP.add,
                )
                osb = op_.tile([P, D], F32, tag="osb")
                nc.scalar.activation(
                    osb[:], big[:],
                    AF.Tanh if gelu_tanh_standin else AF.Gelu, scale=1.0,
                )
                nc.sync.dma_start(out_dram[rows, :], osb[:])

            stage_offsets = [
                (stage_a, 0), (stage_c, 1), (stage_d, 2), (stage_cp, 3),
                (stage_e, 4), (stage_e_dve, 6), (stage_f1, 7), (stage_f2, 8),
            ]
            n_stages = 9
            for i in range(n_chunks + n_stages - 1):
                for fn, off in stage_offsets:
                    c = i - off
                    if 0 <= c < n_chunks:
                        fn(c)

    nc.compile()
    return nc


_CACHE = {}


def _get_nc():
    if "nc" not in _CACHE:
        _CACHE["nc"] = build_nc()
    return _CACHE["nc"]


def _make_in_maps(inputs):
    x = np.ascontiguousarray(
        np.asarray(inputs["expert_features"], np.float32)
    ).astype(ml_dtypes.bfloat16)
    assert x.shape == (B, L, E, D)
    shards = x.reshape(N_CORES, ROWS_PER_CORE, D)

    wn = np.asarray(inputs["W_neighbor"], np.float64)
    ws = np.asarray(inputs["W_self"], np.float64)
    bn = np.asarray(inputs["b_neighbor"], np.float64)
    bs = np.asarray(inputs["b_self"], np.float64)
    wmsg = np.asarray(inputs["w_msg"], np.float32)
    bmsg = float(np.asarray(inputs["b_msg"], np.float32).reshape(-1)[0])
    adjL = np.asarray(inputs["adjacency_logits"], np.float64)

    nd = D // P

    def pack_t(w):
        # [128, 4*512]: block dc holds W^T[128dc:128(dc+1), :]
        wt = np.ascontiguousarray(w.T)  # [d, o]
        return (
            wt.reshape(nd, P, D).transpose(1, 0, 2).reshape(P, nd * D)
        ).astype(ml_dtypes.bfloat16)

    wab = np.empty((P, 2 * D), np.float32)
    wab[:, 0:D] = wmsg[:D]
    wab[:, D:] = wmsg[D:]
    bias_rep = np.ascontiguousarray(
        np.broadcast_to((bn + bs).astype(np.float32), (P, D))
    ).astype(np.float32)

    # block-diag half adjacency: h8[p,q] = 0.5*sigmoid(adjL[q,p])*(p!=q)
    adjs = 1.0 / (1.0 + np.exp(-adjL))
    h8 = 0.5 * adjs.T * (1.0 - np.eye(E))
    hadj = np.kron(np.eye(P // E), h8).astype(ml_dtypes.bfloat16)

    ab6i = np.zeros((P, P), np.float32)
    for j in range(NG):
        ab6i[:, 64 * j + 1] = 1.0
        ab6i[:, 64 * j + 2] = bmsg
        ab6i[:, 64 * j + 32] = 1.0
        ab6i[:, 64 * j + 34] = 1.0
    ab6i = np.tile(ab6i, (1, NSG)).astype(ml_dtypes.bfloat16)

    scati = np.zeros((P, NSC * 2 * P), np.float32)
    eye = np.eye(P, dtype=np.float32)
    for i in range(NSC):
        scati[:, i * 2 * P + P : (i + 1) * 2 * P] = eye
    scati = scati.astype(ml_dtypes.bfloat16)

    params = {
        "wnt": pack_t(wn),
        "wst": pack_t(ws),
        "wab": wab.astype(ml_dtypes.bfloat16),
        "bias_rep": bias_rep,
        "hadj": np.ascontiguousarray(hadj),
        "ab6i": np.ascontiguousarray(ab6i),
        "scati": np.ascontiguousarray(scati),
    }
    return [dict(expert_features=shards[c], **params) for c in range(N_CORES)]


def _run(inputs, trace=False):
    nc = _get_nc()
    in_maps = _make_in_maps(inputs)
    res = bass_utils.run_bass_kernel_spmd(
        nc, in_maps, core_ids=list(range(N_CORES)), trace=trace
    )
    out = np.stack([res.results[c]["out"] for c in range(N_CORES)], axis=0)
    return out.reshape(B, L, E, D), res


def kernel(**inputs):
    out, _ = _run(inputs, trace=False)
    return out


# revision 27
# speedup vs baseline: 1.0039x; 1.0038x over previous
"""ExpertGraphConv Trainium2 kernel (bf16 pipeline, host-precomputed params).

Full inputs in, full output out. Shards batch dim (B=8) across 8 NeuronCores;
params replicated. Each core processes 2048 tokens x 8 experts = 16384 rows.

Math per (token t, expert i):
  adj = sigmoid(adjacency_logits); wa, wb = w_msg[:D], w_msg[D:]
  a[t,i] = x[t,i] . wa ; b[t,j] = x[t,j] . wb
  strength[t,i,j] = adj[i,j] * sigmoid(a[t,i]+b[t,j]+b_msg) * (i != j)
  msg[t,i] = sum_j strength[t,i,j] x[t,j]
  out = gelu(msg @ Wn^T + x @ Ws^T + bn + bs)

Host side precomputes everything cheap: x cast to bf16 (the value path was
bf16 anyway -- halves x HBM traffic), Wn^T/Ws^T pre-transposed bf16 packed
as [128, 4*512], wa/wb/bias broadcast to 128 partitions, the block-diagonal
half-adjacency [128,128], and the ab6 constant pattern. Device setup is a
handful of DMAs issued on the scalar engine's DGE queue (parallel with the
sync queue's first x loads) + one identity.

Per 128-row chunk (16 tokens x 8 experts), 9-stage software pipeline; the
score stages work on 2-chunk pairs so every PE operand is produced >= 1
full iteration before the PE instruction that consumes it:
  A(c):   DMA xbf (bf16 natural layout)
  C(c):   DVE a/b row-sums (fused mult + row-sum accum) -> ab6 pair tile
          (chunk j of the pair at cols 64j+0 (b) / 64j+33 (a); 64-col
          stride because engine PSUM/SBUF partition accesses must start
          32-aligned)
  D(g):   per pair: PE transpose of the [128,128] ab6 pair tile
  CP(g):  per pair: assemble the K=35 fused-outer operands from abT PSUM
          (DVE: lhs35 rows 0:3/32:35 <- b-groups; ACT: rhs35 <- a-groups;
          rows 3..31 stay zero) -- walrus requires fmap/weight at the
          same SB base partition, hence the copies
  E(g):   per pair: ONE K=35 PE matmul -> scores for both chunks
          [128,256], ACT tanh (sigmoid identity)
  E2(c):  DVE strength: sb = (th+1)*hadj (= sigmoid * adj^T, masked)
  F1(c):  PE 4 bf16 matmuls (lhsT=xbf d-slices, rhs=[Sblk|I128b]) yields
          msg^T and x^T together in PSUM; ACT copy -> bf16 zt
  F2(c):  8 accumulating bf16 matmuls (msg^T vs wnT, x^T vs wsT;
          start=True on the first -- no bias matmul), DVE adds bn+bs into
          PSUM in-place, native Gelu from PSUM, DMA out.
Steady-state engine busy per chunk (measured): PE 2.37us, DVE 2.31us,
ACT 2.26us -- near-balanced saturation; PE floor is the bf16 GEMM work
(F2 4096c + F1 1024c at 2.4GHz).
Dead ends (measured/verified, do not revisit):
 - DMA-XBAR transposes for x^T: ~145ns/16x128-tile descriptor generation
   on the issuing sequencer (~4.6us/chunk), regardless of batching.
 - fp8 e4m3 DoubleRow for the GEMMs: rel err 3.5-5e-2 > 2e-2 tolerance
   (numpy-validated), even with hi/lo residual splits.
 - gpsimd scalar_tensor_tensor has no Pool lowering in walrus
   (tensor_tensor does, but gpsimd shares its SBUF port with DVE and
   ~1.3us software latency stalls the chain -- measured slower).
 - Matmul operands at different SB base partitions are rejected by
   walrus even with tile_position.
 - Engine (DVE/ACT) partition accesses must start at 0/32/64/96.
"""

import math
from contextlib import ExitStack

import ml_dtypes
import numpy as np

import concourse.bacc as bacc
import concourse.mybir as mybir
import concourse.tile as tile
from concourse import bass_utils
from concourse.masks import make_identity

F32 = mybir.dt.float32
BF16 = mybir.dt.bfloat16
AF = mybir.ActivationFunctionType
OP = mybir.AluOpType

B, L, E, D = 8, 2048, 8, 512
N_CORES = 8
P = 128
ROWS_PER_CORE = (B // N_CORES) * L * E  # 16384
NSC = 8   # rotation depth of the scat rotating buffers
NG = 2    # chunks per ab6 transpose group (64-col stride: PSUM reads must
          # start at 32-aligned partitions, so per-chunk groups sit at
          # 64j and 64j+32)
NSG = 4   # rotation depth of ab6 group buffers


def build_nc(n_rows=ROWS_PER_CORE, gelu_tanh_standin=False):
    assert n_rows % P == 0
    n_chunks = n_rows // P
    nd = D // P  # 4 d-chunks

    nc = bacc.Bacc(
        "TRN2", target_bir_lowering=False, debug=False, num_devices=N_CORES
    )

    x_dram = nc.dram_tensor("expert_features", [n_rows, D], BF16, kind="ExternalInput").ap()
    wnt_dram = nc.dram_tensor("wnt", [P, nd * D], BF16, kind="ExternalInput").ap()
    wst_dram = nc.dram_tensor("wst", [P, nd * D], BF16, kind="ExternalInput").ap()
    wab_dram = nc.dram_tensor("wab", [P, 2 * D], BF16, kind="ExternalInput").ap()
    bias_dram = nc.dram_tensor("bias_rep", [P, D], F32, kind="ExternalInput").ap()
    hadj_dram = nc.dram_tensor("hadj", [P, P], BF16, kind="ExternalInput").ap()
    ab6i_dram = nc.dram_tensor("ab6i", [P, NSG * P], BF16, kind="ExternalInput").ap()
    scati_dram = nc.dram_tensor("scati", [P, NSC * 2 * P], BF16, kind="ExternalInput").ap()
    out_dram = nc.dram_tensor("out", [n_rows, D], F32, kind="ExternalOutput").ap()

    with tile.TileContext(nc) as tc, ExitStack() as ctx:
        # ---- static SBUF tensors ----
        I128b = nc.alloc_sbuf_tensor("c_I128b", [P, P], BF16).ap()
        wab_all = nc.alloc_sbuf_tensor("c_wab", [P, 2 * D], BF16).ap()
        wa_rep = wab_all[:, 0:D]
        wb_rep = wab_all[:, D : 2 * D]
        bias_rep = nc.alloc_sbuf_tensor("c_bias_rep", [P, D], F32).ap()
        hadj_blk = nc.alloc_sbuf_tensor("c_hadj_blk", [P, P], BF16).ap()
        wnT_all = nc.alloc_sbuf_tensor("c_wnT", [P, nd * D], BF16).ap()
        wsT_all = nc.alloc_sbuf_tensor("c_wsT", [P, nd * D], BF16).ap()
        wnT = [wnT_all[:, dc * D : (dc + 1) * D] for dc in range(nd)]
        wsT = [wsT_all[:, dc * D : (dc + 1) * D] for dc in range(nd)]
        scat_all = nc.alloc_sbuf_tensor("c_scat", [P, NSC * 2 * P], BF16).ap()
        scat = [scat_all[:, i * 2 * P : (i + 1) * 2 * P] for i in range(NSC)]
        # 2-chunk groups; per chunk j: col 64j+0=b, +1=1, +2=bmsg, +32=1,
        # +33=a, +34=1
        ab6_all = nc.alloc_sbuf_tensor("c_ab6", [P, NSG * P], BF16).ap()
        ab6_g = [ab6_all[:, i * P : (i + 1) * P] for i in range(NSG)]
        # K=35 fused outer-pair operands (rows 3..31 stay zero; per-pair
        # copies rewrite rows 0:3 and 32:35 only)
        lhs35_s = [
            nc.alloc_sbuf_tensor(f"c_lhs35_{i}", [35, P], BF16).ap()
            for i in range(NSG)
        ]
        rhs35_s = [
            nc.alloc_sbuf_tensor(f"c_rhs35_{i}", [35, 2 * P], BF16).ap()
            for i in range(NSG)
        ]

        make_identity(nc, I128b)
        for t_ in lhs35_s:
            nc.gpsimd.memset(t_, 0.0)
        for t_ in rhs35_s:
            nc.gpsimd.memset(t_, 0.0)

        # ---- setup: DMA loads of host-precomputed params (scalar queue),
        # ordered by when the pipeline first needs them ----
        nc.scalar.dma_start(wab_all, wab_dram[:])
        nc.scalar.dma_start(ab6_all, ab6i_dram[:])
        nc.scalar.dma_start(hadj_blk, hadj_dram[:])
        nc.scalar.dma_start(scat_all, scati_dram[:])
        nc.scalar.dma_start(wnT_all, wnt_dram[:])
        nc.scalar.dma_start(wsT_all, wst_dram[:])
        nc.scalar.dma_start(bias_rep, bias_dram[:])

        # ---- main loop (software-pipelined, 7-stage skew) ----
        with (
            tc.tile_pool(name="xp", bufs=10) as xp,
            tc.tile_pool(name="sc", bufs=4) as scp,
            tc.tile_pool(name="small", bufs=3) as smp,
            tc.tile_pool(name="mid", bufs=3) as midp,
            tc.tile_pool(name="ztp", bufs=6) as ztp,
            tc.tile_pool(name="op", bufs=5) as op_,
            tc.tile_pool(name="ps_t", bufs=2, space="PSUM") as ps_t,
            tc.tile_pool(name="ps_o", bufs=1, space="PSUM") as ps_o,
            tc.tile_pool(name="ps_c", bufs=1, space="PSUM") as ps_c,
            tc.tile_pool(name="ps_b", bufs=3, space="PSUM") as ps_b,
        ):
            xins = {}
            abTs = {}
            cps = {}
            zts = {}
            ths = {}

            def stage_a(c):
                rows = slice(c * P, (c + 1) * P)
                xin = xp.tile([P, D], BF16, tag="xin")
                nc.sync.dma_start(xin[:], x_dram[rows, :])
                xins[c] = xin

            def stage_c(c):
                ab6 = ab6_g[(c // NG) % NSG]
                j = c % NG
                xbf = xins[c]
                scr = scp.tile([P, D], BF16, tag="scr")
                nc.vector.scalar_tensor_tensor(
                    out=scr[:], in0=xbf[:], scalar=0.0, in1=wb_rep,
                    op0=OP.bypass, op1=OP.mult, accum_out=ab6[:, 64 * j : 64 * j + 1],
                )
                scr2 = scp.tile([P, D], BF16, tag="scr")
                nc.vector.scalar_tensor_tensor(
                    out=scr2[:], in0=xbf[:], scalar=0.0, in1=wa_rep,
                    op0=OP.bypass, op1=OP.mult,
                    accum_out=ab6[:, 64 * j + 33 : 64 * j + 34],
                )

            def stage_d(c):
                # fires on the last chunk of each group (or the final chunk)
                if not (c % NG == NG - 1 or c == n_chunks - 1):
                    return
                g = c // NG
                abT = ps_t.tile([P, P], BF16, tag="abT")
                nc.tensor.transpose(abT[:], ab6_g[g % NSG][:], I128b)
                abTs[g] = abT

            def stage_cp(c):
                # operand copies one iteration ahead of the fused outer
                # matmul, so the PE never waits on same-iteration
                # cross-engine operands. Split DVE/ACT to balance load.
                if c % NG != NG - 1:
                    return
                g = c // NG
                abT = abTs.pop(g)
                l35 = lhs35_s[g % NSG]
                r35 = rhs35_s[g % NSG]
                nc.vector.tensor_copy(l35[0:3, :], abT[0:3, :])
                nc.vector.tensor_copy(l35[32:35, :], abT[64:67, :])
                nc.scalar.copy(r35[0:3, 0:P], abT[32:35, :])
                nc.scalar.copy(r35[32:35, P : 2 * P], abT[96:99, :])

            def stage_e(c):
                # K=35 fused outer for the chunk pair: scores for chunk 2g
                # in cols 0:128, chunk 2g+1 in cols 128:256
                if c % NG != NG - 1:
                    return
                g = c // NG
                outer = ps_o.tile([P, 2 * P], F32, tag="outer")
                nc.tensor.matmul(
                    outer[:], lhsT=lhs35_s[g % NSG][:], rhs=rhs35_s[g % NSG][:]
                )
                th = midp.tile([P, 2 * P], BF16, tag="th")
                nc.scalar.activation(th[:], outer[:], AF.Tanh, scale=0.5)
                ths[g] = th

            def stage_e_dve(c):
                # sb = (th+1)*hadj = sigmoid*adjT masked
                g, j = c // NG, c % NG
                th = ths[g]
                if j == NG - 1:
                    ths.pop(g)
                sb = scat[c % NSC]
                nc.vector.scalar_tensor_tensor(
                    out=sb[:, 0:P], in0=th[:, j * P : (j + 1) * P], scalar=1.0,
                    in1=hadj_blk, op0=OP.add, op1=OP.mult,
                )

            def stage_f1(c):
                xin = xins.pop(c)
                sb = scat[c % NSC]
                cmb = ps_c.tile([P, 2 * P * nd], F32, tag="cmb")
                for dc in range(nd):
                    nc.tensor.matmul(
                        cmb[:, 2 * P * dc : 2 * P * (dc + 1)],
                        lhsT=xin[:, dc * P : (dc + 1) * P],
                        rhs=sb[:],
                    )
                zt = ztp.tile([P, 2 * P * nd], BF16, tag="zt")
                nc.scalar.copy(zt[:], cmb[:])
                zts[c] = zt

            def stage_f2(c):
                rows = slice(c * P, (c + 1) * P)
                zt = zts.pop(c)
                big = ps_b.tile([P, D], F32, tag="big")
                for dc in range(nd):
                    nc.tensor.matmul(
                        big[:],
                        lhsT=zt[:, 2 * P * dc : 2 * P * dc + P],
                        rhs=wnT[dc],
                        start=(dc == 0),
                        stop=False,
                    )
                for dc in range(nd):
                    nc.tensor.matmul(
                        big[:],
                        lhsT=zt[:, 2 * P * dc + P : 2 * P * (dc + 1)],
                        rhs=wsT[dc],
                        start=False,
                        stop=(dc == nd - 1),
                    )
                # bias into PSUM in-place, then gelu reads the biased PSUM
                nc.vector.scalar_tensor_tensor(
                    out=big[:], in0=big[:], scalar=0.0, in1=bias_rep,
                    op0=OP.bypass, op1=OP.add,
                )
                osb = op_.tile([P, D], F32, tag="osb")
                nc.scalar.activation(
                    osb[:], big[:],
                    AF.Tanh if gelu_tanh_standin else AF.Gelu, scale=1.0,
                )
                nc.sync.dma_start(out_dram[rows, :], osb[:])

            stage_offsets = [
                (stage_a, 0), (stage_c, 1), (stage_d, 2), (stage_cp, 3),
                (stage_e, 4), (stage_e_dve, 6), (stage_f1, 7), (stage_f2, 8),
            ]
            n_stages = 9
            for i in range(n_chunks + n_stages - 1):
                for fn, off in stage_offsets:
                    c = i - off
                    if 0 <= c < n_chunks:
                        fn(c)

    nc.compile()
    return nc


_CACHE = {}


def _get_nc():
    if "nc" not in _CACHE:
        _CACHE["nc"] = build_nc()
    return _CACHE["nc"]


def _make_in_maps(inputs):
    x = np.ascontiguousarray(
        np.asarray(inputs["expert_features"], np.float32)
    ).astype(ml_dtypes.bfloat16)
    assert x.shape == (B, L, E, D)
    shards = x.reshape(N_CORES, ROWS_PER_CORE, D)

    wn = np.asarray(inputs["W_neighbor"], np.float64)
    ws = np.asarray(inputs["W_self"], np.float64)
    bn = np.asarray(inputs["b_neighbor"], np.float64)
    bs = np.asarray(inputs["b_self"], np.float64)
    wmsg = np.asarray(inputs["w_msg"], np.float32)
    bmsg = float(np.asarray(inputs["b_msg"], np.float32).reshape(-1)[0])
    adjL = np.asarray(inputs["adjacency_logits"], np.float64)

    nd = D // P

    def pack_t(w):
        # [128, 4*512]: block dc holds W^T[128dc:128(dc+1), :]
        wt = np.ascontiguousarray(w.T)  # [d, o]
        return (
            wt.reshape(nd, P, D).transpose(1, 0, 2).reshape(P, nd * D)
        ).astype(ml_dtypes.bfloat16)

    wab = np.empty((P, 2 * D), np.float32)
    wab[:, 0:D] = wmsg[:D]
    wab[:, D:] = wmsg[D:]
    bias_rep = np.ascontiguousarray(
        np.broadcast_to((bn + bs).astype(np.float32), (P, D))
    ).astype(np.float32)

    # block-diag half adjacency: h8[p,q] = 0.5*sigmoid(adjL[q,p])*(p!=q)
    adjs = 1.0 / (1.0 + np.exp(-adjL))
    h8 = 0.5 * adjs.T * (1.0 - np.eye(E))
    hadj = np.kron(np.eye(P // E), h8).astype(ml_dtypes.bfloat16)

    ab6i = np.zeros((P, P), np.float32)
    for j in range(NG):
        ab6i[:, 64 * j + 1] = 1.0
        ab6i[:, 64 * j + 2] = bmsg
        ab6i[:, 64 * j + 32] = 1.0
        ab6i[:, 64 * j + 34] = 1.0
    ab6i = np.tile(ab6i, (1, NSG)).astype(ml_dtypes.bfloat16)

    scati = np.zeros((P, NSC * 2 * P), np.float32)
    eye = np.eye(P, dtype=np.float32)
    for i in range(NSC):
        scati[:, i * 2 * P + P : (i + 1) * 2 * P] = eye
    scati = scati.astype(ml_dtypes.bfloat16)

    params = {
        "wnt": pack_t(wn),
        "wst": pack_t(ws),
        "wab": wab.astype(ml_dtypes.bfloat16),
        "bias_rep": bias_rep,
        "hadj": np.ascontiguousarray(hadj),
        "ab6i": np.ascontiguousarray(ab6i),
        "scati": np.ascontiguousarray(scati),
    }
    return [dict(expert_features=shards[c], **params) for c in range(N_CORES)]


def _run(inputs, trace=False):
    nc = _get_nc()
    in_maps = _make_in_maps(inputs)
    res = bass_utils.run_bass_kernel_spmd(
        nc, in_maps, core_ids=list(range(N_CORES)), trace=trace
    )
    out = np.stack([res.results[c]["out"] for c in range(N_CORES)], axis=0)
    return out.reshape(B, L, E, D), res


def kernel(**inputs):
    out, _ = _run(inputs, trace=False)
    return out
